# revision 4
# baseline (speedup 1.0000x reference)
"""VQ codebook assignment (ApplyKmeans) on 8 Trainium2 NeuronCores.

tokens[n] = argmin_k ||x_n - c_k||^2
          = argmax_k (x_n.c_k - Cnorm_k/2)        (||x_n||^2 constant per row)

Data-parallel: x sharded along N across 8 cores, C/Cnorm replicated.

Per core (16384 rows, 128 row-tiles of 128 rows), a 4-engine pipeline:
  PE   : 8 accumulating fp16 matmuls per tile -> PSUM f32 [128,300]
         (start=True on j=0; ~127ns/matmul issue cadence, LDWEIGHTS hidden)
  ACT  : copy PSUM -> SBUF f32 (ScalarE sits closest to PSUM)
  GPS  : add -Cnorm/2 bias (SBUF->SBUF; GPSIMD has no PSUM port)
  DVE  : max8 + max_index on the biased SBUF scores -> first-occurrence
         argmax per row (f32 throughout: fp16 scores flip ~640 tokens)
Per-tile engine busy ~= PE 1027 / DVE ~975 / GPS ~670 / ACT ~490 ns, so the
PE is the sole steady-state bottleneck and the argmax pipeline drains right
behind it instead of pegging the PSUM pool (the old 7us tail).

Startup: the PE p-state ramp (1.2GHz until ~3us of continuous work) is
absorbed by warmup matmuls on zeroed SBUF while the first data DMAs land.
Group 0 arrives as 8 per-tile DMAs (tile-major host layout) on the sync
queue so tile 0 can start after 256KB instead of 2MB; constants + odd
groups ride the ACT queue, even groups the sync queue (two HWDGE queues
roughly double aggregate x bandwidth and halve first-data latency).

Row interleaving: row-tile t holds rows {p*128 + t}, so the token buffer
[p, t] DMAs out contiguously in original row order.

Walrus only lowers one sync wait per instruction; _hoist_excess_waits
moves Tile's extra waits onto same-engine no-ops at the same program
point. The lane-pool hook gives ACT-issued and sync-issued DMAs disjoint
completion lanes so lane-order WAW waits can't serialize one queue behind
the other.
"""

import os
import sys

import numpy as np

if "/opt/trn_rl_repo" not in sys.path:
    sys.path.insert(0, "/opt/trn_rl_repo")

import concourse.bass as bass
import concourse.mybir as mybir
import concourse.tile_sem_assignment as _tsa
from concourse.bass_utils import run_bass_kernel_spmd
from concourse.tile import TileContext

_tsa.NUM_HWDGE_SEMS = int(os.environ.get("KM_HW_LANES", "8"))

# Give each HWDGE ring (SP-issued vs ACT-issued DMAs) a disjoint pool of
# completion lanes. Tile's global round-robin otherwise interleaves the
# two rings onto shared lanes, and the lane-order WAW waits then falsely
# serialize one ring behind the other.
_orig_assign_tick = _tsa.TileClockTick._assign_tick


def _assign_tick_lanepools(self, inst):
    try:
        if isinstance(inst, _tsa.DMAInst) and inst.engine != mybir.EngineType.Pool:
            if not hasattr(self, "_lane_ctr"):
                self._lane_ctr = {}
            eng = inst.engine
            n = _tsa.NUM_HWDGE_SEMS
            half = max(1, n // 2)
            pool = (
                list(range(0, half))
                if eng == mybir.EngineType.Activation
                else list(range(half, n))
            )
            c = self._lane_ctr.get(eng, 0)
            self.next_hw_dma_idx = pool[c % len(pool)]
            self._lane_ctr[eng] = c + 1
    except Exception:
        pass
    return _orig_assign_tick(self, inst)


_tsa.TileClockTick._assign_tick = _assign_tick_lanepools

P = 128
D = 1024
K = 300
NCORES = 8
ROWS = 16384            # rows per core
TILES = ROWS // P       # 128 row-tiles per core
GROUPS = 32             # DMA groups per core (1 group = 1 MB fp16)
TPG = TILES // GROUPS   # 8 row-tiles per group
DCH = D // P            # 8 contraction chunks

F16 = mybir.dt.float16
F32 = mybir.dt.float32
I32 = mybir.dt.int32
U32 = mybir.dt.uint32

# Set by kernel() so test.py can read profiling info.
LAST_RESULT = None


def _ensure_ntff_hook():
    """Install antenv.axon_hooks shim so trace=True works under axon."""
    try:
        from antenv.axon_hooks import get_axon_ntff_profile_hook  # noqa: F401

        return
    except ImportError:
        pass
    import types

    import antenv

    try:
        from trn_agent_boot.trn_boot import _ntff_profile_via_ctypes
    except ImportError:
        return
    mod = types.ModuleType("antenv.axon_hooks")
    _hook = [None]
    mod.set_axon_ntff_profile_hook = lambda h: _hook.__setitem__(0, h)
    mod.get_axon_ntff_profile_hook = lambda: _hook[0]
    sys.modules["antenv.axon_hooks"] = mod
    antenv.axon_hooks = mod
    so = "/opt/axon/libaxon_pjrt.so"
    if os.path.exists(so):
        mod.set_axon_ntff_profile_hook(_ntff_profile_via_ctypes(so))


def build_nc() -> bass.Bass:
    nc = bass.Bass()

    warm = int(os.environ.get("KM_WARM", "12"))
    xbufs = int(os.environ.get("KM_XBUFS", "4"))

    xg = nc.declare_dram_parameter("xg", [GROUPS, P, TPG * DCH * P], F16, isOutput=False)
    cons = nc.declare_dram_parameter("cons", [P, DCH * K], F16, isOutput=False)
    biasf = nc.declare_dram_parameter("biasf", [P, K], F32, isOutput=False)
    out = nc.declare_dram_parameter("out", [P, TILES], I32, isOutput=True)

    OSL = 16        # token output slice, in tiles
    OSL_TAIL = 4    # finer slices for the last OSL tiles -> shorter tail

    with TileContext(nc) as tc:
        with (
            tc.tile_pool(name="const", bufs=1) as constp,
            tc.tile_pool(name="xp0", bufs=TPG) as xp0,
            tc.tile_pool(name="xp", bufs=xbufs) as xp,
            tc.tile_pool(name="vr", bufs=4) as vrp,
            tc.tile_pool(name="vl", bufs=4) as vlp,
            tc.tile_pool(name="mx", bufs=8) as mxp,
            tc.tile_pool(name="psum", bufs=7, space="PSUM") as psp,
            tc.tile_pool(name="wps", bufs=1, space="PSUM") as wps,
            tc.tile_pool(name="outp", bufs=1) as outp,
        ):
            # constants on the ACT HWDGE queue (parallel with x on sync):
            # bias first (cheap), then C chunk 0 (gates tile-0 matmul j=0),
            # then the rest.
            bft = constp.tile([P, K], F32)
            nc.scalar.dma_start(out=bft[:], in_=biasf[:])
            cons_t = constp.tile([P, DCH * K], F16)
            nc.scalar.dma_start(out=cons_t[:, :K], in_=cons[:, :K])
            nc.scalar.dma_start(out=cons_t[:, K:], in_=cons[:, K:])
            ctiles = [cons_t[:, j * K : (j + 1) * K] for j in range(DCH)]

            # PE warmup: the tensor engine needs ~3us of continuous work to
            # leave the 1.2GHz p-state. Burn it on zeroed SBUF into a scratch
            # PSUM bank while the first x / C DMAs are still in flight.
            if warm:
                wtile = constp.tile([P, 384], F16)
                nc.gpsimd.memset(wtile[:], 0.0)
                wpsum = wps.tile([P, K], F32)
                for _ in range(warm):
                    nc.tensor.matmul(
                        wpsum[:, :256], lhsT=wtile[:, :128], rhs=wtile[:, 128:],
                        start=True, stop=True,
                    )

            # group 0 arrives tile-by-tile so the PE can start after 256KB
            xch0 = []
            for tl in range(TPG):
                cb = xp0.tile([P, DCH, P], F16, name="x0tile")
                nc.sync.dma_start(
                    out=cb[:],
                    in_=xg[0, :, tl * DCH * P : (tl + 1) * DCH * P].rearrange(
                        "p (j q) -> p j q", j=DCH
                    ),
                )
                xch0.append(cb)

            idxbuf = outp.tile([P, TILES, 8], U32)
            tokbuf = outp.tile([P, TILES], I32)

            for g in range(GROUPS):
                if g == 0:
                    chunk = lambda j, tl: xch0[tl][:, j, :]
                else:
                    # stripe whole-group loads across both HWDGE queues
                    xbuf = xp.tile([P, TPG, DCH, P], F16, name="xgrp")
                    eng = nc.scalar if (g % 2 == 1) else nc.sync
                    eng.dma_start(
                        out=xbuf[:],
                        in_=xg[g].rearrange("p (t j q) -> p t j q", t=TPG, j=DCH),
                    )
                    chunk = lambda j, tl, xbuf=xbuf: xbuf[:, tl, j, :]
                for tl in range(TPG):
                    t = g * TPG + tl
                    psum = psp.tile([P, K], F32)
                    for j in range(DCH):
                        nc.tensor.matmul(
                            psum[:],
                            lhsT=chunk(j, tl),
                            rhs=ctiles[j][:],
                            start=(j == 0),
                            stop=(j == DCH - 1),
                        )
                    # drain PSUM on ACT (closest engine to PSUM), bias on
                    # GPSIMD (no PSUM port, but SBUF adds are fine), then
                    # DVE finds the argmax from SBUF
                    vr = vrp.tile([P, K], F32)
                    nc.scalar.copy(out=vr[:], in_=psum[:])
                    vl = vlp.tile([P, K], F32)
                    nc.gpsimd.tensor_tensor(
                        out=vl[:], in0=vr[:], in1=bft[:], op=mybir.AluOpType.add
                    )
                    mx = mxp.tile([P, 8], F32)
                    nc.vector.max(out=mx[:], in_=vl[:])
                    nc.vector.max_index(
                        out=idxbuf[:, t, :], in_max=mx[:], in_values=vl[:]
                    )
                    # stream tokens out on the ACT queue; finer slices at the
                    # end so the last tile's out-DMA chain is short
                    osl = OSL_TAIL if t >= TILES - OSL else OSL
                    if (t + 1) % osl == 0:
                        s = t + 1 - osl
                        nc.gpsimd.tensor_copy(
                            out=tokbuf[:, s : t + 1], in_=idxbuf[:, s : t + 1, 0]
                        )
                        nc.scalar.dma_start(
                            out=out[:, s : t + 1], in_=tokbuf[:, s : t + 1]
                        )

    _hoist_excess_waits(nc)
    return nc


def _hoist_excess_waits(nc: bass.Bass, max_waits: int = 1):
    """Hoist excess sync waits onto no-op drains inserted just before.

    Walrus's codegen caps embedded sync waits per instruction (1 for
    DIRECT2D DMAs and CTRL ops), but Tile can attach several (slot-reuse
    WAR + lane WAW, or the kernel-tail drain waiting on every proc).
    A same-engine drain immediately before the instruction blocks the
    sequencer at the same program point, so semantics are unchanged.
    """
    n = 0
    for f in nc.m.functions:
        for blk in f.blocks:
            insts = blk.instructions
            i = 0
            while i < len(insts):
                inst = insts[i]
                si = inst.sync_info
                if si and si.on_wait and len(si.on_wait) > max_waits:
                    waits = list(si.on_wait)
                    si.on_wait = waits[-max_waits:]
                    inst.sync_info = si
                    pre = []
                    for j in range(0, len(waits) - max_waits, max_waits):
                        nd = mybir.InstNoOp(name=f"I-wsplit{n}", ins=[], outs=[])
                        n += 1
                        nd.engine = inst.engine
                        nsi = type(si)(
                            on_wait=waits[j : j + max_waits], on_update=[]
                        )
                        nd.sync_info = nsi
                        try:
                            nc.register_instruction(nd, overwrite=True)
                        except Exception:
                            pass
                        pre.append(nd)
                    for k, nd in enumerate(pre):
                        insts.insert(i + k, nd)
                    i += len(pre)
                i += 1


def make_in_maps(x: np.ndarray, C: np.ndarray, Cnorm: np.ndarray):
    x16 = x.astype(np.float16)
    C16 = C.astype(np.float16).reshape(DCH, P, K)

    cons = np.ascontiguousarray(C16.transpose(1, 0, 2)).reshape(P, DCH * K)
    biasf = np.broadcast_to(
        (-0.5 * Cnorm.reshape(K)).astype(np.float32), (P, K)
    ).copy()

    in_maps = []
    for c in range(NCORES):
        xs = x16[c * ROWS : (c + 1) * ROWS]
        # row r = p*128 + g*TPG + tl ; col = j*128 + q
        xr = xs.reshape(P, GROUPS, TPG, DCH, P)          # [p, g, tl, j, q]
        xgc = np.ascontiguousarray(xr.transpose(1, 4, 2, 3, 0))  # [g, q, tl, j, p]
        in_maps.append(
            {
                "xg": xgc.reshape(GROUPS, P, TPG * DCH * P),
                "cons": cons,
                "biasf": biasf,
            }
        )
    return in_maps


_NC_CACHE = {}


def kernel(x, C, Cnorm, b, t):
    global LAST_RESULT
    x = np.asarray(x)
    C = np.asarray(C)
    Cnorm = np.asarray(Cnorm)

    key = 0
    if key not in _NC_CACHE:
        _NC_CACHE[key] = build_nc()
    nc = _NC_CACHE[key]

    in_maps = make_in_maps(x, C, Cnorm)
    trace = bool(int(os.environ.get("KM_TRACE", "0")))
    if trace:
        _ensure_ntff_hook()
    res = run_bass_kernel_spmd(
        nc, in_maps, core_ids=list(range(NCORES)), trace=trace
    )
    LAST_RESULT = res

    shards = [res.results[c]["out"].reshape(-1) for c in range(NCORES)]
    tokens = np.concatenate(shards).astype(np.int32)
    return tokens.reshape(int(b), int(t))


# revision 8
# speedup vs baseline: 1.0198x; 1.0198x over previous
"""VQ codebook assignment (ApplyKmeans) on 8 Trainium2 NeuronCores.

tokens[n] = argmin_k ||x_n - c_k||^2
          = argmax_k (x_n.c_k - Cnorm_k/2)        (||x_n||^2 constant per row)

Data-parallel: x sharded along N across 8 cores, C/Cnorm replicated.

Per core (16384 rows, 128 row-tiles of 128 rows), a 4-engine pipeline:
  PE   : 8 accumulating fp16 matmuls per tile -> PSUM f32 [128,300]
         (start=True on j=0; ~127ns/matmul issue cadence, LDWEIGHTS hidden)
  ACT  : copy PSUM -> SBUF f32 (ScalarE sits closest to PSUM)
  GPS  : add -Cnorm/2 bias (SBUF->SBUF; GPSIMD has no PSUM port)
  DVE  : max8 + max_index on the biased SBUF scores -> first-occurrence
         argmax per row (f32 throughout: fp16 scores flip ~640 tokens)
Per-tile engine busy ~= PE 1027 / DVE ~975 / GPS ~670 / ACT ~490 ns, so the
PE is the sole steady-state bottleneck and the argmax pipeline drains right
behind it instead of pegging the PSUM pool (the old 7us tail).

Startup: the PE p-state ramp (1.2GHz until ~3us of continuous work) is
absorbed by warmup matmuls on zeroed SBUF while the first data DMAs land.
Group 0 arrives as 8 per-tile DMAs (tile-major host layout) on the sync
queue so tile 0 can start after 256KB instead of 2MB; constants + odd
groups ride the ACT queue, even groups the sync queue (two HWDGE queues
roughly double aggregate x bandwidth and halve first-data latency).

Row interleaving: row-tile t holds rows {p*128 + t}, so the token buffer
[p, t] DMAs out contiguously in original row order.

Walrus only lowers one sync wait per instruction; _hoist_excess_waits
moves Tile's extra waits onto same-engine no-ops at the same program
point. The lane-pool hook gives ACT-issued and sync-issued DMAs disjoint
completion lanes so lane-order WAW waits can't serialize one queue behind
the other.
"""

import os
import sys

import numpy as np

if "/opt/trn_rl_repo" not in sys.path:
    sys.path.insert(0, "/opt/trn_rl_repo")

import concourse.bass as bass
import concourse.mybir as mybir
import concourse.tile_sem_assignment as _tsa
from concourse.bass_utils import run_bass_kernel_spmd
from concourse.tile import TileContext

_tsa.NUM_HWDGE_SEMS = int(os.environ.get("KM_HW_LANES", "8"))

# Give each HWDGE ring (SP-issued vs ACT-issued DMAs) a disjoint pool of
# completion lanes. Tile's global round-robin otherwise interleaves the
# two rings onto shared lanes, and the lane-order WAW waits then falsely
# serialize one ring behind the other.
_orig_assign_tick = _tsa.TileClockTick._assign_tick


def _assign_tick_lanepools(self, inst):
    try:
        if isinstance(inst, _tsa.DMAInst) and inst.engine != mybir.EngineType.Pool:
            if not hasattr(self, "_lane_ctr"):
                self._lane_ctr = {}
            eng = inst.engine
            n = _tsa.NUM_HWDGE_SEMS
            half = max(1, n // 2)
            pool = (
                list(range(0, half))
                if eng == mybir.EngineType.Activation
                else list(range(half, n))
            )
            c = self._lane_ctr.get(eng, 0)
            self.next_hw_dma_idx = pool[c % len(pool)]
            self._lane_ctr[eng] = c + 1
    except Exception:
        pass
    return _orig_assign_tick(self, inst)


_tsa.TileClockTick._assign_tick = _assign_tick_lanepools

P = 128
D = 1024
K = 300
NCORES = 8
ROWS = 16384            # rows per core
TILES = ROWS // P       # 128 row-tiles per core
GROUPS = 32             # DMA groups per core (1 group = 1 MB fp16)
TPG = TILES // GROUPS   # 8 row-tiles per group
DCH = D // P            # 8 contraction chunks

F16 = mybir.dt.float16
F32 = mybir.dt.float32
I32 = mybir.dt.int32
U32 = mybir.dt.uint32

# Set by kernel() so test.py can read profiling info.
LAST_RESULT = None


def _ensure_ntff_hook():
    """Install antenv.axon_hooks shim so trace=True works under axon."""
    try:
        from antenv.axon_hooks import get_axon_ntff_profile_hook  # noqa: F401

        return
    except ImportError:
        pass
    import types

    import antenv

    try:
        from trn_agent_boot.trn_boot import _ntff_profile_via_ctypes
    except ImportError:
        return
    mod = types.ModuleType("antenv.axon_hooks")
    _hook = [None]
    mod.set_axon_ntff_profile_hook = lambda h: _hook.__setitem__(0, h)
    mod.get_axon_ntff_profile_hook = lambda: _hook[0]
    sys.modules["antenv.axon_hooks"] = mod
    antenv.axon_hooks = mod
    so = "/opt/axon/libaxon_pjrt.so"
    if os.path.exists(so):
        mod.set_axon_ntff_profile_hook(_ntff_profile_via_ctypes(so))


def build_nc() -> bass.Bass:
    nc = bass.Bass()

    warm = int(os.environ.get("KM_WARM", "16"))
    xbufs = int(os.environ.get("KM_XBUFS", "4"))

    xg = nc.declare_dram_parameter("xg", [GROUPS, P, TPG * DCH * P], F16, isOutput=False)
    cons = nc.declare_dram_parameter("cons", [P, DCH * K], F16, isOutput=False)
    biasf = nc.declare_dram_parameter("biasf", [P, K], F32, isOutput=False)
    out = nc.declare_dram_parameter("out", [P, TILES], I32, isOutput=True)

    OSL = 16        # token output slice, in tiles
    OSL_TAIL = 4    # finer slices for the last OSL tiles -> shorter tail

    with TileContext(nc) as tc:
        with (
            tc.tile_pool(name="const", bufs=1) as constp,
            tc.tile_pool(name="xp0", bufs=TPG) as xp0,
            tc.tile_pool(name="xp", bufs=xbufs) as xp,
            tc.tile_pool(name="vr", bufs=4) as vrp,
            tc.tile_pool(name="vl", bufs=4) as vlp,
            tc.tile_pool(name="mx", bufs=8) as mxp,
            tc.tile_pool(name="psum", bufs=7, space="PSUM") as psp,
            tc.tile_pool(name="wps", bufs=1, space="PSUM") as wps,
            tc.tile_pool(name="outp", bufs=1) as outp,
        ):
            # constants on the ACT HWDGE queue (parallel with x on sync):
            # bias first (cheap), then C chunk 0 (gates tile-0 matmul j=0),
            # then the rest.
            cons_t = constp.tile([P, DCH * K], F16)
            nc.scalar.dma_start(out=cons_t[:, :K], in_=cons[:, :K])
            bft = constp.tile([P, K], F32)
            nc.scalar.dma_start(out=bft[:], in_=biasf[:])
            nc.scalar.dma_start(out=cons_t[:, K:], in_=cons[:, K:])
            ctiles = [cons_t[:, j * K : (j + 1) * K] for j in range(DCH)]

            # PE warmup: the tensor engine needs ~3us of continuous work to
            # leave the 1.2GHz p-state. Burn it on zeroed SBUF into a scratch
            # PSUM bank while the first x / C DMAs are still in flight.
            if warm:
                wtile = constp.tile([P, 384], F16)
                nc.vector.memset(wtile[:], 0.0)
                wpsum = wps.tile([P, K], F32)
                for _ in range(warm):
                    nc.tensor.matmul(
                        wpsum[:, :256], lhsT=wtile[:, :128], rhs=wtile[:, 128:],
                        start=True, stop=True,
                    )

            # group 0 arrives tile-by-tile so the PE can start after 256KB
            xch0 = []
            for tl in range(TPG):
                cb = xp0.tile([P, DCH, P], F16, name="x0tile")
                nc.sync.dma_start(
                    out=cb[:],
                    in_=xg[0, :, tl * DCH * P : (tl + 1) * DCH * P].rearrange(
                        "p (j q) -> p j q", j=DCH
                    ),
                )
                xch0.append(cb)

            idxbuf = outp.tile([P, TILES, 8], U32)
            tokbuf = outp.tile([P, TILES], I32)

            for g in range(GROUPS):
                if g == 0:
                    chunk = lambda j, tl: xch0[tl][:, j, :]
                else:
                    # stripe whole-group loads across both HWDGE queues
                    xbuf = xp.tile([P, TPG, DCH, P], F16, name="xgrp")
                    eng = nc.scalar if (g % 2 == 1) else nc.sync
                    eng.dma_start(
                        out=xbuf[:],
                        in_=xg[g].rearrange("p (t j q) -> p t j q", t=TPG, j=DCH),
                    )
                    chunk = lambda j, tl, xbuf=xbuf: xbuf[:, tl, j, :]
                for tl in range(TPG):
                    t = g * TPG + tl
                    psum = psp.tile([P, K], F32)
                    for j in range(DCH):
                        nc.tensor.matmul(
                            psum[:],
                            lhsT=chunk(j, tl),
                            rhs=ctiles[j][:],
                            start=(j == 0),
                            stop=(j == DCH - 1),
                        )
                    # drain PSUM on ACT (closest engine to PSUM), bias on
                    # GPSIMD (no PSUM port, but SBUF adds are fine), then
                    # DVE finds the argmax from SBUF
                    vr = vrp.tile([P, K], F32)
                    nc.scalar.copy(out=vr[:], in_=psum[:])
                    vl = vlp.tile([P, K], F32)
                    nc.gpsimd.tensor_tensor(
                        out=vl[:], in0=vr[:], in1=bft[:], op=mybir.AluOpType.add
                    )
                    mx = mxp.tile([P, 8], F32)
                    nc.vector.max(out=mx[:], in_=vl[:])
                    nc.vector.max_index(
                        out=idxbuf[:, t, :], in_max=mx[:], in_values=vl[:]
                    )
                    # stream tokens out on the ACT queue; finer slices at the
                    # end so the last tile's out-DMA chain is short
                    osl = OSL_TAIL if t >= TILES - OSL else OSL
                    if (t + 1) % osl == 0:
                        s = t + 1 - osl
                        nc.gpsimd.tensor_copy(
                            out=tokbuf[:, s : t + 1], in_=idxbuf[:, s : t + 1, 0]
                        )
                        # sync queue: an out-DMA issue waits on the token
                        # copy, and on the ACT queue that wait would block
                        # the PSUM-drain copies behind it (head-of-line)
                        nc.sync.dma_start(
                            out=out[:, s : t + 1], in_=tokbuf[:, s : t + 1]
                        )

    _hoist_excess_waits(nc)
    return nc


def _hoist_excess_waits(nc: bass.Bass, max_waits: int = 1):
    """Hoist excess sync waits onto no-op drains inserted just before.

    Walrus's codegen caps embedded sync waits per instruction (1 for
    DIRECT2D DMAs and CTRL ops), but Tile can attach several (slot-reuse
    WAR + lane WAW, or the kernel-tail drain waiting on every proc).
    A same-engine drain immediately before the instruction blocks the
    sequencer at the same program point, so semantics are unchanged.
    """
    n = 0
    for f in nc.m.functions:
        for blk in f.blocks:
            insts = blk.instructions
            i = 0
            while i < len(insts):
                inst = insts[i]
                si = inst.sync_info
                if si and si.on_wait and len(si.on_wait) > max_waits:
                    waits = list(si.on_wait)
                    si.on_wait = waits[-max_waits:]
                    inst.sync_info = si
                    pre = []
                    for j in range(0, len(waits) - max_waits, max_waits):
                        nd = mybir.InstNoOp(name=f"I-wsplit{n}", ins=[], outs=[])
                        n += 1
                        nd.engine = inst.engine
                        nsi = type(si)(
                            on_wait=waits[j : j + max_waits], on_update=[]
                        )
                        nd.sync_info = nsi
                        try:
                            nc.register_instruction(nd, overwrite=True)
                        except Exception:
                            pass
                        pre.append(nd)
                    for k, nd in enumerate(pre):
                        insts.insert(i + k, nd)
                    i += len(pre)
                i += 1


def make_in_maps(x: np.ndarray, C: np.ndarray, Cnorm: np.ndarray):
    x16 = x.astype(np.float16)
    C16 = C.astype(np.float16).reshape(DCH, P, K)

    cons = np.ascontiguousarray(C16.transpose(1, 0, 2)).reshape(P, DCH * K)
    biasf = np.broadcast_to(
        (-0.5 * Cnorm.reshape(K)).astype(np.float32), (P, K)
    ).copy()

    in_maps = []
    for c in range(NCORES):
        xs = x16[c * ROWS : (c + 1) * ROWS]
        # row r = p*128 + g*TPG + tl ; col = j*128 + q
        xr = xs.reshape(P, GROUPS, TPG, DCH, P)          # [p, g, tl, j, q]
        xgc = np.ascontiguousarray(xr.transpose(1, 4, 2, 3, 0))  # [g, q, tl, j, p]
        in_maps.append(
            {
                "xg": xgc.reshape(GROUPS, P, TPG * DCH * P),
                "cons": cons,
                "biasf": biasf,
            }
        )
    return in_maps


_NC_CACHE = {}


def kernel(x, C, Cnorm, b, t):
    global LAST_RESULT
    x = np.asarray(x)
    C = np.asarray(C)
    Cnorm = np.asarray(Cnorm)

    key = 0
    if key not in _NC_CACHE:
        _NC_CACHE[key] = build_nc()
    nc = _NC_CACHE[key]

    in_maps = make_in_maps(x, C, Cnorm)
    trace = bool(int(os.environ.get("KM_TRACE", "0")))
    if trace:
        _ensure_ntff_hook()
    res = run_bass_kernel_spmd(
        nc, in_maps, core_ids=list(range(NCORES)), trace=trace
    )
    LAST_RESULT = res

    shards = [res.results[c]["out"].reshape(-1) for c in range(NCORES)]
    tokens = np.concatenate(shards).astype(np.int32)
    return tokens.reshape(int(b), int(t))
